# revision 21
# baseline (speedup 1.0000x reference)
"""Trainium2 Bass kernel for nn_Attention_81793357185069.

4-group attention: N=16, L=M=1024, in/param dim 512, planes 512, out 2048.
Strategy: data-parallel over batch N across 8 NeuronCores (2 batches/core),
zero collectives. All matmuls run in bf16 (1 cycle/row on PE vs 4 for fp32)
with fp32 PSUM accumulation.

Layout trick: everything is computed in "transposed" space so the PE
contraction dim always lands on partitions with zero on-chip transposes:
  - host pre-transposes activations X -> X^T (c, t) and weights W -> W^T
  - Q^T/K^T (planes, t) and V (t, planes) come straight out of projections
  - scores S^T (l, m) = (K^T_g).T @ Q^T_g per group
  - softmax has no max-subtraction (scores are bounded ~ N(0, 0.5^2)) and
    normalization is deferred: P = exp(S^T) (bf16), denominators via a
    3-level pairwise-add tree on DVE (bf16, reduces the 8 l-tiles to one
    partial tile) followed by a single ones-matmul pair on PE for the final
    128-partition reduction + broadcast.  This removes 14 of the 16
    denominator matmuls per unit a pure ones-matmul scheme needs.
  - sv (d, m) = V_g.T @ P accumulated over l-tiles, scaled by 1/denom on
    PSUM evacuation (reciprocal via the fast approx custom-DVE op)
  - out (m, o) = sv_all.T @ Wproj^T; ATTN_SCALE and biases fold host-side.

All PSUM tiles are [128, 1024] (two adjacent banks); matmuls write bank-
aligned [128, 512] halves and evacuations/activations process the full
[128, 1024] in one instruction, halving ACT/DVE per-instruction overhead.
Output is stored bf16 (host casts to f32): halves the output DMA, which is
bandwidth-saturated at the kernel tail.

Scheduling (the PE queue is strictly in-order, so emission order is
everything):
  - per unit: 16 score MMs + 16 SV MMs; SV lags 4 j-steps behind scores so
    ACT exp latency never blocks; the DVE add-tree is woven in as the exps
    complete.
  - each unit's denominator MMs + reciprocal + sv-normalize are DEFERRED
    into the next unit's score stream so the PE never waits on the DVE
    tree at a unit boundary.
  - filler chunks (QKV projections of the other batch, output projection
    of the previous batch) are interleaved one (half-)chunk per j-step so
    the per-j PE work always exceeds the exp rate.
  - the first unit is woven directly into the prologue (its fillers are
    the batch-0 V chunks), which hides the input-DMA ramp.
"""

import math

import ml_dtypes
import numpy as np

import concourse.bass as bass
import concourse.mybir as mybir
import concourse.tile as tile
from concourse import bacc
from concourse.bass_utils import run_bass_kernel_spmd

N_CORES = 8
N = 16
B = N // N_CORES  # batches per core
T = 1024  # L == M
C = 512  # in/param dim
P = 512  # planes
O = 2048  # out dim
G = 4  # groups
D = P // G  # 128 group planes
ATTN_SCALE = P ** (-0.5)
EQ_SCALE = 1.0 / math.sqrt(C)

CT = C // 128  # 4 contraction tiles
TT = T // 128  # 8 l/m tiles
MCH = T // 512  # 2 moving chunks of 512

BF = mybir.dt.bfloat16
F32 = mybir.dt.float32

_CACHE: dict = {}


def _emit(tc, has_cout):
    nc = tc.nc
    AF = mybir.ActivationFunctionType

    xt_op = nc.dram_tensor("xt_op", [B, 128, CT, T], BF, kind="ExternalInput").ap()
    xt_att = nc.dram_tensor("xt_att", [B, 128, CT, T], BF, kind="ExternalInput").ap()
    wqT = nc.dram_tensor("wqT", [C, P], BF, kind="ExternalInput").ap()
    wkT = nc.dram_tensor("wkT", [C, P], BF, kind="ExternalInput").ap()
    wvT = nc.dram_tensor("wvT", [128, CT * P], BF, kind="ExternalInput").ap()
    wpT = nc.dram_tensor("wpT", [128, CT * O], BF, kind="ExternalInput").ap()
    qb = nc.dram_tensor("qb", [128, G], F32, kind="ExternalInput").ap()
    kb = nc.dram_tensor("kb", [128, G], F32, kind="ExternalInput").ap()
    coutb = (
        nc.dram_tensor("coutb", [128, O], F32, kind="ExternalInput").ap()
        if has_cout
        else None
    )
    out = nc.dram_tensor("out", [B, T, O], BF, kind="ExternalOutput").ap()

    with (
        tc.tile_pool(name="const", bufs=1) as const,
        tc.tile_pool(name="xt", bufs=1) as xtp,
        tc.tile_pool(name="qkv", bufs=2) as qkvp,
        tc.tile_pool(name="pt", bufs=2) as ptp,
        tc.tile_pool(name="tr", bufs=2) as trp,
        tc.tile_pool(name="rdn", bufs=1) as rdnp,
        tc.tile_pool(name="svt", bufs=2) as svtp,
        tc.tile_pool(name="ost", bufs=4) as ostp,
        tc.tile_pool(name="ps", bufs=3, space="PSUM") as psp,
        tc.tile_pool(name="pvp", bufs=1, space="PSUM") as pvp,
    ):
        # ---- constants / weights ----
        wq_s = [const.tile([128, P], BF, tag=f"wq{i}", name=f"wq{i}") for i in range(CT)]
        wk_s = [const.tile([128, P], BF, tag=f"wk{i}", name=f"wk{i}") for i in range(CT)]
        wvall = const.tile([128, CT * P], BF, tag="wvall", name="wvall")
        wv_s = [wvall[:, i * P : (i + 1) * P] for i in range(CT)]
        wpall = const.tile([128, CT * O], BF, tag="wpall", name="wpall")
        wp_s = [wpall[:, i * O : (i + 1) * O] for i in range(CT)]
        qball = const.tile([128, G], F32, tag="qball", name="qball")
        qb_s = [qball[:, g : g + 1] for g in range(G)]
        kball = const.tile([128, G], F32, tag="kball", name="kball")
        kb_s = [kball[:, g : g + 1] for g in range(G)]
        cout_s = const.tile([128, O], F32, tag="cout", name="cout") if has_cout else None
        ones_s = const.tile([128, 128], BF, tag="ones", name="ones")

        xo0_s = [xtp.tile([128, T], BF, tag=f"xo{i}", name=f"xo{i}_0") for i in range(CT)]
        xa0_s = [xtp.tile([128, T], BF, tag=f"xa{i}", name=f"xa{i}_0") for i in range(CT)]
        xo1all = xtp.tile([128, CT * T], BF, tag="xo1", name="xo1all")
        xa1all = xtp.tile([128, CT * T], BF, tag="xa1", name="xa1all")
        xo_s = {0: xo0_s, 1: [xo1all[:, i * T : (i + 1) * T] for i in range(CT)]}
        xa_s = {0: xa0_s, 1: [xa1all[:, i * T : (i + 1) * T] for i in range(CT)]}
        qT_s = {b: [qkvp.tile([128, T], BF, tag=f"q{g}", name=f"qT{g}_{b}") for g in range(G)] for b in range(B)}
        kT_s = {b: [qkvp.tile([128, T], BF, tag=f"k{g}", name=f"kT{g}_{b}") for g in range(G)] for b in range(B)}
        # V for batch b: [128, TT*512] with l-tile j at cols [j*512, (j+1)*512)
        vall = {b: qkvp.tile([128, TT * P], BF, tag="vall", name=f"vall_{b}") for b in range(B)}
        svT = {b: [svtp.tile([128, T], BF, tag=f"s{g}", name=f"svT{g}_{b}") for g in range(G)] for b in range(B)}

        # DMA order = need order, interleaved at tile grain and balanced
        # across the two HWDGE queues (scalar: Q-side + V weights + xo1;
        # sync: K-side + xa1 + proj weights) so batch-0 inputs land ASAP.
        nc.scalar.dma_start(xo0_s[0][:, 0:512], xt_op[0, :, 0, 0:512])
        nc.scalar.dma_start(wq_s[0][:], wqT[0:128, :])
        nc.scalar.dma_start(xo0_s[0][:, 512:1024], xt_op[0, :, 0, 512:1024])
        for i in range(1, CT):
            nc.scalar.dma_start(xo0_s[i][:], xt_op[0, :, i, :])
            nc.scalar.dma_start(wq_s[i][:], wqT[i * 128 : (i + 1) * 128, :])
        nc.scalar.dma_start(qball[:], qb[:, :])
        nc.sync.dma_start(xa0_s[0][:, 0:512], xt_att[0, :, 0, 0:512])
        nc.sync.dma_start(wk_s[0][:], wkT[0:128, :])
        nc.sync.dma_start(xa0_s[0][:, 512:1024], xt_att[0, :, 0, 512:1024])
        for i in range(1, CT):
            nc.sync.dma_start(xa0_s[i][:], xt_att[0, :, i, :])
            nc.sync.dma_start(wk_s[i][:], wkT[i * 128 : (i + 1) * 128, :])
        nc.sync.dma_start(kball[:], kb[:, :])
        nc.sync.dma_start(wvall[:], wvT[:, :])
        nc.vector.memset(ones_s[:], 1.0)
        # batch-1 activations before proj weights: fillers need them first
        nc.sync.dma_start(xo1all[:], xt_op[1, :, :, :])
        nc.sync.dma_start(xa1all[:], xt_att[1, :, :, :])
        nc.sync.dma_start(wpall[:], wpT[:, :])
        if has_cout:
            nc.sync.dma_start(cout_s[:], coutb[:, :])

        # ---- PE warm-up: dummy matmuls while the first inputs stream in.
        # The PE clock is HAM-throttled to 1.2 GHz until it has been busy
        # for a ~3.4us activity window; burn that window on ones x ones
        # matmuls during the otherwise PE-idle DMA ramp so the real stream
        # starts at 2.4 GHz.
        warm = psp.tile([128, 1024], F32, tag="ps", name="warmps")
        for r in range(44):
            nc.tensor.matmul(
                warm[:, (r % 8) * 128 : (r % 8 + 1) * 128],
                ones_s[:],
                ones_s[:],
                start=True,
                stop=True,
            )

        # ---- chunk emitters: one [128,1024] 2-bank PSUM group + 1 evac ----
        # Each emitter can be split into two half-bursts of 4 MMs; the evac
        # instruction rides with the second half.  eng: 'v' (DVE) / 's' (ACT).
        def chunk_q(b, g, eng, half=None):
            if half is None or half == 0:
                chunk_q.ps = psp.tile([128, 1024], F32, tag="ps", name=f"psq{g}_{b}")
            pq = chunk_q.ps
            cts = range(CT) if half is None else (range(2) if half == 0 else range(2, CT))
            for ct in cts:
                for mch in range(MCH):
                    nc.tensor.matmul(
                        pq[:, mch * 512 : (mch + 1) * 512],
                        wq_s[ct][:, g * 128 : (g + 1) * 128],
                        xo_s[b][ct][:, mch * 512 : (mch + 1) * 512],
                        start=(ct == 0),
                        stop=(ct == CT - 1),
                    )
            if half is None or half == 1:
                if eng == "v":
                    nc.vector.tensor_scalar_add(qT_s[b][g][:], pq[:], qb_s[g][:])
                else:
                    nc.scalar.activation(qT_s[b][g][:], pq[:], AF.Identity, bias=qb_s[g][:])

        def chunk_k(b, g, eng, half=None):
            if half is None or half == 0:
                chunk_k.ps = psp.tile([128, 1024], F32, tag="ps", name=f"psk{g}_{b}")
            pk = chunk_k.ps
            cts = range(CT) if half is None else (range(2) if half == 0 else range(2, CT))
            for ct in cts:
                for mch in range(MCH):
                    nc.tensor.matmul(
                        pk[:, mch * 512 : (mch + 1) * 512],
                        wk_s[ct][:, g * 128 : (g + 1) * 128],
                        xa_s[b][ct][:, mch * 512 : (mch + 1) * 512],
                        start=(ct == 0),
                        stop=(ct == CT - 1),
                    )
            if half is None or half == 1:
                if eng == "v":
                    nc.vector.tensor_scalar_add(kT_s[b][g][:], pk[:], kb_s[g][:])
                else:
                    nc.scalar.activation(kT_s[b][g][:], pk[:], AF.Identity, bias=kb_s[g][:])

        def chunk_v(b, jj, eng, half=None):
            if half is None or half == 0:
                chunk_v.ps = psp.tile([128, 1024], F32, tag="ps", name=f"psv{jj}_{b}")
            pv = chunk_v.ps
            cts = range(CT) if half is None else (range(2) if half == 0 else range(2, CT))
            for ct in cts:
                for jh in range(2):
                    j = jj * 2 + jh
                    nc.tensor.matmul(
                        pv[:, jh * 512 : (jh + 1) * 512],
                        xa_s[b][ct][:, j * 128 : (j + 1) * 128],
                        wv_s[ct][:],
                        start=(ct == 0),
                        stop=(ct == CT - 1),
                    )
            if half is None or half == 1:
                dst = vall[b][:, jj * 1024 : (jj + 1) * 1024]
                if eng == "v":
                    nc.vector.tensor_copy(dst, pv[:])
                else:
                    nc.scalar.copy(dst, pv[:])

        def chunk_proj(b, mt, oh, eng, half=None):
            if half is None or half == 0:
                chunk_proj.ps = psp.tile([128, 1024], F32, tag="ps", name=f"pso{mt}{oh}_{b}")
            po = chunk_proj.ps
            gs = range(G) if half is None else (range(2) if half == 0 else range(2, G))
            for g in gs:
                for oc in range(2):
                    nc.tensor.matmul(
                        po[:, oc * 512 : (oc + 1) * 512],
                        svT[b][g][:, mt * 128 : (mt + 1) * 128],
                        wp_s[g][:, oh * 1024 + oc * 512 : oh * 1024 + (oc + 1) * 512],
                        start=(g == 0),
                        stop=(g == G - 1),
                    )
            if half is None or half == 1:
                ost = ostp.tile([128, 1024], BF, tag="ost", name=f"ost{mt}{oh}_{b}")
                if has_cout:
                    nc.vector.tensor_add(ost[:], po[:], cout_s[:, oh * 1024 : (oh + 1) * 1024])
                elif eng == "v":
                    nc.vector.tensor_copy(ost[:], po[:])
                else:
                    nc.scalar.copy(ost[:], po[:])
                nc.sync.dma_start(
                    out[b, mt * 128 : (mt + 1) * 128, oh * 1024 : (oh + 1) * 1024], ost[:]
                )

        def halves(f):
            """Split one chunk emitter into two slot-sized bursts."""
            return [lambda: f(0), lambda: f(1)]

        # ---- attention unit ----
        def emit_unit(b, g, slots, carry_in):
            """slots: list of 8 filler-callable-lists, one per j-step.
            carry_in: deferred denominator work of the previous unit.
            Returns this unit's deferred work (den MMs + recip + svmul)."""
            pT = [ptp.tile([128, T], BF, tag=f"p{j}", name=f"pT{j}_{g}_{b}") for j in range(TT)]
            t1 = [trp.tile([128, T], BF, tag=f"t1{p}", name=f"t1{p}_{g}_{b}") for p in range(4)]
            t2 = [trp.tile([128, T], BF, tag=f"t2{p}", name=f"t2{p}_{g}_{b}") for p in range(2)]
            t3 = trp.tile([128, T], BF, tag="t3", name=f"t3_{g}_{b}")
            pv2 = pvp.tile([128, 1024], F32, tag="pv", name=f"pssv_{g}_{b}")
            rden = rdnp.tile([128, T], F32, tag="rd", name=f"rden{g}_{b}")

            def s_step(j):
                ps = psp.tile([128, 1024], F32, tag="ps", name=f"pss{j}_{g}_{b}")
                for mch in range(MCH):
                    nc.tensor.matmul(
                        ps[:, mch * 512 : (mch + 1) * 512],
                        kT_s[b][g][:, j * 128 : (j + 1) * 128],
                        qT_s[b][g][:, mch * 512 : (mch + 1) * 512],
                        start=True,
                        stop=True,
                    )
                nc.scalar.activation(pT[j][:], ps[:], AF.Exp)

            def sv_step(j):
                for mch in range(MCH):
                    nc.tensor.matmul(
                        pv2[:, mch * 512 : (mch + 1) * 512],
                        vall[b][:, j * 512 + g * 128 : j * 512 + (g + 1) * 128],
                        pT[j][:, mch * 512 : (mch + 1) * 512],
                        start=(j == 0),
                        stop=(j == TT - 1),
                    )

            def run(fs):
                for f in fs:
                    f()

            s_step(0)
            run(slots[0])
            s_step(1)
            run(slots[1])
            for f in carry_in[:1]:
                f()  # previous unit's SV tail (exp(6)/exp(7) latency cover)
            s_step(2)
            nc.vector.tensor_add(t1[0][:], pT[0][:], pT[1][:])
            for f in carry_in[1:]:
                f()  # previous unit's denominator + normalize
            run(slots[2])
            s_step(3)
            run(slots[3])
            s_step(4)
            nc.vector.tensor_add(t1[1][:], pT[2][:], pT[3][:])
            run(slots[4])
            s_step(5)
            run(slots[5])
            sv_step(0)
            s_step(6)
            nc.vector.tensor_add(t1[2][:], pT[4][:], pT[5][:])
            nc.vector.tensor_add(t2[0][:], t1[0][:], t1[1][:])
            run(slots[6])
            sv_step(1)
            s_step(7)
            nc.vector.tensor_add(t1[3][:], pT[6][:], pT[7][:])
            run(slots[7])
            sv_step(2)
            sv_step(3)
            sv_step(4)

            def sv_tail():
                sv_step(5)
                sv_step(6)
                sv_step(7)
                nc.vector.tensor_add(t2[1][:], t1[2][:], t1[3][:])
                nc.vector.tensor_add(t3[:], t2[0][:], t2[1][:])

            def deferred():
                pden = psp.tile([128, 1024], F32, tag="ps", name=f"psd_{g}_{b}")
                for mch in range(MCH):
                    nc.tensor.matmul(
                        pden[:, mch * 512 : (mch + 1) * 512],
                        ones_s[:],
                        t3[:, mch * 512 : (mch + 1) * 512],
                        start=True,
                        stop=True,
                    )
                nc.vector.reciprocal_approx_fast(rden[:], pden[:])
                nc.vector.tensor_mul(svT[b][g][:], pv2[:], rden[:])

            return [sv_tail, deferred]

        # ---- whole-kernel schedule ----
        def fq(b, g, e):
            return halves(lambda h: chunk_q(b, g, e, h))

        def fk(b, g, e):
            return halves(lambda h: chunk_k(b, g, e, h))

        def fv(b, jj, e):
            return halves(lambda h: chunk_v(b, jj, e, h))

        def fp(b, a, e):
            return halves(lambda h: chunk_proj(b, a // 2, a % 2, e, h))

        def whole(hs):
            h0, h1 = hs
            return [lambda: (h0(), h1())[0]]

        # prologue: Q00 first (its inputs lead the scalar DMA queue), then
        # Q01/Q02 to cover the K-side DMA latency, then K00.
        chunk_q(0, 0, "s")
        chunk_q(0, 1, "v")
        chunk_q(0, 2, "s")
        chunk_k(0, 0, "v")

        # unit (0,0) carries the rest of the batch-0 QKV (whole chunks: the
        # early ones double as DMA-ramp cover, the V chunks feed its own SV)
        carry = emit_unit(0, 0, [
            whole(fq(0, 3, "s")),
            whole(fk(0, 1, "v")),
            whole(fk(0, 2, "s")),
            whole(fk(0, 3, "v")),
            whole(fv(0, 0, "s")),
            whole(fv(0, 1, "v")),
            whole(fv(0, 2, "s")),
            whole(fv(0, 3, "v")),
        ], [])

        def half_slots(pairs):
            """4 chunk-halves pairs -> 8 slots."""
            out = []
            for h0, h1 in pairs:
                out.append([h0])
                out.append([h1])
            return out

        carry = emit_unit(0, 1, half_slots([
            fq(1, 0, "s"), fq(1, 1, "v"), fq(1, 2, "v"), fq(1, 3, "v"),
        ]), carry)
        carry = emit_unit(0, 2, half_slots([
            fk(1, 0, "s"), fk(1, 1, "v"), fk(1, 2, "v"), fk(1, 3, "v"),
        ]), carry)
        carry = emit_unit(0, 3, half_slots([
            fv(1, 0, "s"), fv(1, 1, "v"), fv(1, 2, "v"), fv(1, 3, "v"),
        ]), carry)

        def whole_late(chunks):
            """4 whole-chunk fillers at slots 3..6: late enough that the
            previous unit's deferred svT (ready ~s3) is available."""
            return [[], [], [], [chunks[0]], [chunks[1]], [chunks[2]], [chunks[3]], []]

        carry = emit_unit(1, 0, whole_late([
            whole(fp(0, 0, "s"))[0], whole(fp(0, 1, "v"))[0],
            whole(fp(0, 2, "s"))[0], whole(fp(0, 3, "v"))[0],
        ]), carry)
        carry = emit_unit(1, 1, whole_late([
            whole(fp(0, 4, "s"))[0], whole(fp(0, 5, "v"))[0],
            whole(fp(0, 6, "s"))[0], whole(fp(0, 7, "v"))[0],
        ]), carry)
        carry = emit_unit(1, 2, whole_late([
            whole(fp(0, 8, "s"))[0], whole(fp(0, 9, "v"))[0],
            whole(fp(0, 10, "s"))[0], whole(fp(0, 11, "v"))[0],
        ]), carry)
        carry = emit_unit(1, 3, whole_late([
            whole(fp(0, 12, "s"))[0], whole(fp(0, 13, "v"))[0],
            whole(fp(0, 14, "s"))[0], whole(fp(0, 15, "v"))[0],
        ]), carry)
        # epilogue: drain the last unit's deferred work, then batch-1 proj
        carry[0]()
        carry[1]()
        for a in range(15):
            chunk_proj(1, a // 2, a % 2, "vs"[a % 2])
        # last chunk: evacuate + DMA in halves so the final output transfer
        # (which nothing can hide) is half-sized
        po = psp.tile([128, 1024], F32, tag="ps", name="pso_last")
        for g in range(G):
            for oc in range(2):
                nc.tensor.matmul(
                    po[:, oc * 512 : (oc + 1) * 512],
                    svT[1][g][:, 7 * 128 : 8 * 128],
                    wp_s[g][:, 1024 + oc * 512 : 1024 + (oc + 1) * 512],
                    start=(g == 0),
                    stop=(g == G - 1),
                )
        ost_l = ostp.tile([128, 1024], BF, tag="ost", name="ost_last")
        for oc in range(2):
            osl = ost_l[:, oc * 512 : (oc + 1) * 512]
            if has_cout:
                nc.vector.tensor_add(osl, po[:, oc * 512 : (oc + 1) * 512],
                                     cout_s[:, 1024 + oc * 512 : 1024 + (oc + 1) * 512])
            elif oc == 0:
                nc.scalar.copy(osl, po[:, 0:512])
            else:
                nc.vector.tensor_copy(osl, po[:, 512:1024])
            nc.sync.dma_start(
                out[1, 7 * 128 : 8 * 128, 1024 + oc * 512 : 1024 + (oc + 1) * 512], osl
            )


def _build(has_cout):
    nc = bacc.Bacc(
        "TRN2", target_bir_lowering=False, debug=False, num_devices=N_CORES
    )
    with tile.TileContext(nc) as tc:
        _emit(tc, has_cout)
    nc.compile()
    return nc


def get_nc(has_cout=False):
    key = ("nc", has_cout)
    if key not in _CACHE:
        _CACHE[key] = _build(has_cout)
    return _CACHE[key]


def prep_inputs(attention, op_param, q_w, q_b, k_w, k_b, v_w, v_b, proj_w, proj_b):
    """Host-side layout prep: fold scales, transpose, cast to bf16, shard."""
    bf16 = ml_dtypes.bfloat16
    f32 = np.float32

    att = np.asarray(attention, f32)
    op = np.asarray(op_param, f32)

    # (n, t, c) -> (n, c, t) -> (n, 128, CT, t), bf16
    def relay_x(x):
        xt = x.transpose(0, 2, 1).reshape(x.shape[0], CT, 128, x.shape[1])
        return np.ascontiguousarray(xt.transpose(0, 2, 1, 3)).astype(bf16)

    xt_att = relay_x(att)
    xt_op = relay_x(op)

    wqT = np.ascontiguousarray(
        (np.asarray(q_w, f32) * (EQ_SCALE * ATTN_SCALE)).T
    ).astype(bf16)
    wkT = np.ascontiguousarray((np.asarray(k_w, f32) * EQ_SCALE).T).astype(bf16)

    def relay_w(wT, ncol):
        # [C, ncol] -> [128, CT*ncol] with ct-tile i at cols [i*ncol,(i+1)*ncol)
        w4 = wT.reshape(CT, 128, ncol).transpose(1, 0, 2).reshape(128, CT * ncol)
        return np.ascontiguousarray(w4).astype(bf16)

    wvT = relay_w(np.asarray(v_w, f32).T * EQ_SCALE, P)
    # proj is also an EqualLinear: weight scale 1/sqrt(PLANES) = EQ_SCALE
    wp_scaled = np.asarray(proj_w, f32) * EQ_SCALE
    wpT = relay_w(wp_scaled.T, O)

    qb2 = np.ascontiguousarray(
        (np.asarray(q_b, f32) * ATTN_SCALE).reshape(G, 128).T
    )
    kb2 = np.ascontiguousarray(np.asarray(k_b, f32).reshape(G, 128).T)
    # sum_l sim = 1, so v_b contributes proj_w @ v_b to every output row
    cout = wp_scaled @ np.asarray(v_b, f32) + np.asarray(proj_b, f32)
    has_cout = bool(np.any(cout != 0.0))
    coutb = np.ascontiguousarray(np.broadcast_to(cout[None, :], (128, O))).astype(f32)

    in_maps = []
    for core in range(N_CORES):
        lo, hi = core * B, (core + 1) * B
        m = {
            "xt_op": np.ascontiguousarray(xt_op[lo:hi]),
            "xt_att": np.ascontiguousarray(xt_att[lo:hi]),
            "wqT": wqT,
            "wkT": wkT,
            "wvT": wvT,
            "wpT": wpT,
            "qb": qb2,
            "kb": kb2,
        }
        if has_cout:
            m["coutb"] = coutb
        in_maps.append(m)
    return in_maps


def run(in_maps, trace=False, **kw):
    has_cout = "coutb" in in_maps[0]
    nc = get_nc(has_cout)
    res = run_bass_kernel_spmd(nc, in_maps, list(range(N_CORES)), trace=trace, **kw)
    return res


def kernel(**inputs) -> np.ndarray:
    in_maps = prep_inputs(**inputs)
    res = run(in_maps)
    out = np.concatenate([res.results[i]["out"] for i in range(N_CORES)], axis=0)
    return out.astype(np.float32)


# revision 22
# speedup vs baseline: 1.0096x; 1.0096x over previous
"""Trainium2 Bass kernel for nn_Attention_81793357185069.

4-group attention: N=16, L=M=1024, in/param dim 512, planes 512, out 2048.
Strategy: data-parallel over batch N across 8 NeuronCores (2 batches/core),
zero collectives. All matmuls run in bf16 (1 cycle/row on PE vs 4 for fp32)
with fp32 PSUM accumulation.

Layout trick: everything is computed in "transposed" space so the PE
contraction dim always lands on partitions with zero on-chip transposes:
  - host pre-transposes activations X -> X^T (c, t) and weights W -> W^T
  - Q^T/K^T (planes, t) and V (t, planes) come straight out of projections
  - scores S^T (l, m) = (K^T_g).T @ Q^T_g per group
  - softmax has no max-subtraction (scores are bounded ~ N(0, 0.5^2)) and
    normalization is deferred: P = exp(S^T) (bf16), denominators via a
    3-level pairwise-add tree on DVE (bf16, reduces the 8 l-tiles to one
    partial tile) followed by a single ones-matmul pair on PE for the final
    128-partition reduction + broadcast.  This removes 14 of the 16
    denominator matmuls per unit a pure ones-matmul scheme needs.
  - sv (d, m) = V_g.T @ P accumulated over l-tiles, scaled by 1/denom on
    PSUM evacuation (reciprocal via the fast approx custom-DVE op)
  - out (m, o) = sv_all.T @ Wproj^T; ATTN_SCALE and biases fold host-side.

All PSUM tiles are [128, 1024] (two adjacent banks); matmuls write bank-
aligned [128, 512] halves and evacuations/activations process the full
[128, 1024] in one instruction, halving ACT/DVE per-instruction overhead.
Output is stored bf16 (host casts to f32): halves the output DMA, which is
bandwidth-saturated at the kernel tail.

Scheduling (the PE queue is strictly in-order, so emission order is
everything):
  - per unit: 16 score MMs + 16 SV MMs; SV lags 4 j-steps behind scores so
    ACT exp latency never blocks; the DVE add-tree is woven in as the exps
    complete.
  - each unit's denominator MMs + reciprocal + sv-normalize are DEFERRED
    into the next unit's score stream so the PE never waits on the DVE
    tree at a unit boundary.
  - filler chunks (QKV projections of the other batch, output projection
    of the previous batch) are interleaved one (half-)chunk per j-step so
    the per-j PE work always exceeds the exp rate.
  - the first unit is woven directly into the prologue (its fillers are
    the batch-0 V chunks), which hides the input-DMA ramp.
"""

import math

import ml_dtypes
import numpy as np

import concourse.bass as bass
import concourse.mybir as mybir
import concourse.tile as tile
from concourse import bacc
from concourse.bass_utils import run_bass_kernel_spmd

N_CORES = 8
N = 16
B = N // N_CORES  # batches per core
T = 1024  # L == M
C = 512  # in/param dim
P = 512  # planes
O = 2048  # out dim
G = 4  # groups
D = P // G  # 128 group planes
ATTN_SCALE = P ** (-0.5)
EQ_SCALE = 1.0 / math.sqrt(C)

CT = C // 128  # 4 contraction tiles
TT = T // 128  # 8 l/m tiles
MCH = T // 512  # 2 moving chunks of 512

BF = mybir.dt.bfloat16
F32 = mybir.dt.float32

_CACHE: dict = {}


def _emit(tc, has_cout):
    nc = tc.nc
    AF = mybir.ActivationFunctionType

    xt_op = nc.dram_tensor("xt_op", [B, 128, CT, T], BF, kind="ExternalInput").ap()
    xt_att = nc.dram_tensor("xt_att", [B, 128, CT, T], BF, kind="ExternalInput").ap()
    wqT = nc.dram_tensor("wqT", [C, P], BF, kind="ExternalInput").ap()
    wkT = nc.dram_tensor("wkT", [C, P], BF, kind="ExternalInput").ap()
    wvT = nc.dram_tensor("wvT", [128, CT * P], BF, kind="ExternalInput").ap()
    wpT = nc.dram_tensor("wpT", [128, CT * O], BF, kind="ExternalInput").ap()
    qb = nc.dram_tensor("qb", [128, G], F32, kind="ExternalInput").ap()
    kb = nc.dram_tensor("kb", [128, G], F32, kind="ExternalInput").ap()
    coutb = (
        nc.dram_tensor("coutb", [128, O], F32, kind="ExternalInput").ap()
        if has_cout
        else None
    )
    out = nc.dram_tensor("out", [B, T, O], BF, kind="ExternalOutput").ap()

    with (
        tc.tile_pool(name="const", bufs=1) as const,
        tc.tile_pool(name="xt", bufs=1) as xtp,
        tc.tile_pool(name="qkv", bufs=2) as qkvp,
        tc.tile_pool(name="pt", bufs=2) as ptp,
        tc.tile_pool(name="tr", bufs=2) as trp,
        tc.tile_pool(name="rdn", bufs=1) as rdnp,
        tc.tile_pool(name="svt", bufs=2) as svtp,
        tc.tile_pool(name="ost", bufs=4) as ostp,
        tc.tile_pool(name="ps", bufs=3, space="PSUM") as psp,
        tc.tile_pool(name="pvp", bufs=1, space="PSUM") as pvp,
    ):
        # ---- constants / weights ----
        wq_s = [const.tile([128, P], BF, tag=f"wq{i}", name=f"wq{i}") for i in range(CT)]
        wk_s = [const.tile([128, P], BF, tag=f"wk{i}", name=f"wk{i}") for i in range(CT)]
        wvall = const.tile([128, CT * P], BF, tag="wvall", name="wvall")
        wv_s = [wvall[:, i * P : (i + 1) * P] for i in range(CT)]
        wpall = const.tile([128, CT * O], BF, tag="wpall", name="wpall")
        wp_s = [wpall[:, i * O : (i + 1) * O] for i in range(CT)]
        qball = const.tile([128, G], F32, tag="qball", name="qball")
        qb_s = [qball[:, g : g + 1] for g in range(G)]
        kball = const.tile([128, G], F32, tag="kball", name="kball")
        kb_s = [kball[:, g : g + 1] for g in range(G)]
        cout_s = const.tile([128, O], F32, tag="cout", name="cout") if has_cout else None
        ones_s = const.tile([128, 128], BF, tag="ones", name="ones")

        xo0_s = [xtp.tile([128, T], BF, tag=f"xo{i}", name=f"xo{i}_0") for i in range(CT)]
        xa0_s = [xtp.tile([128, T], BF, tag=f"xa{i}", name=f"xa{i}_0") for i in range(CT)]
        xo1all = xtp.tile([128, CT * T], BF, tag="xo1", name="xo1all")
        xa1all = xtp.tile([128, CT * T], BF, tag="xa1", name="xa1all")
        xo_s = {0: xo0_s, 1: [xo1all[:, i * T : (i + 1) * T] for i in range(CT)]}
        xa_s = {0: xa0_s, 1: [xa1all[:, i * T : (i + 1) * T] for i in range(CT)]}
        qT_s = {b: [qkvp.tile([128, T], BF, tag=f"q{g}", name=f"qT{g}_{b}") for g in range(G)] for b in range(B)}
        kT_s = {b: [qkvp.tile([128, T], BF, tag=f"k{g}", name=f"kT{g}_{b}") for g in range(G)] for b in range(B)}
        # V for batch b: [128, TT*512] with l-tile j at cols [j*512, (j+1)*512)
        vall = {b: qkvp.tile([128, TT * P], BF, tag="vall", name=f"vall_{b}") for b in range(B)}
        svT = {b: [svtp.tile([128, T], BF, tag=f"s{g}", name=f"svT{g}_{b}") for g in range(G)] for b in range(B)}

        # DMA order = need order, interleaved at tile grain and balanced
        # across the two HWDGE queues (scalar: Q-side + V weights + xo1;
        # sync: K-side + xa1 + proj weights) so batch-0 inputs land ASAP.
        nc.scalar.dma_start(xo0_s[0][:, 0:512], xt_op[0, :, 0, 0:512])
        nc.scalar.dma_start(wq_s[0][:], wqT[0:128, :])
        nc.scalar.dma_start(xo0_s[0][:, 512:1024], xt_op[0, :, 0, 512:1024])
        for i in range(1, CT):
            nc.scalar.dma_start(xo0_s[i][:], xt_op[0, :, i, :])
            nc.scalar.dma_start(wq_s[i][:], wqT[i * 128 : (i + 1) * 128, :])
        nc.scalar.dma_start(qball[:], qb[:, :])
        nc.sync.dma_start(xa0_s[0][:, 0:512], xt_att[0, :, 0, 0:512])
        nc.sync.dma_start(wk_s[0][:], wkT[0:128, :])
        nc.sync.dma_start(xa0_s[0][:, 512:1024], xt_att[0, :, 0, 512:1024])
        for i in range(1, CT):
            nc.sync.dma_start(xa0_s[i][:], xt_att[0, :, i, :])
            nc.sync.dma_start(wk_s[i][:], wkT[i * 128 : (i + 1) * 128, :])
        nc.sync.dma_start(kball[:], kb[:, :])
        nc.sync.dma_start(wvall[:], wvT[:, :])
        nc.vector.memset(ones_s[:], 1.0)
        # batch-1 activations before proj weights: fillers need them first
        nc.sync.dma_start(xo1all[:], xt_op[1, :, :, :])
        nc.sync.dma_start(xa1all[:], xt_att[1, :, :, :])
        nc.sync.dma_start(wpall[:], wpT[:, :])
        if has_cout:
            nc.sync.dma_start(cout_s[:], coutb[:, :])

        # ---- PE warm-up: dummy matmuls while the first inputs stream in.
        # The PE clock is HAM-throttled to 1.2 GHz until it has been busy
        # for a ~3.4us activity window; burn that window on ones x ones
        # matmuls during the otherwise PE-idle DMA ramp so the real stream
        # starts at 2.4 GHz.
        warm = psp.tile([128, 1024], F32, tag="ps", name="warmps")
        for r in range(44):
            nc.tensor.matmul(
                warm[:, (r % 8) * 128 : (r % 8 + 1) * 128],
                ones_s[:],
                ones_s[:],
                start=True,
                stop=True,
            )

        # ---- chunk emitters: one [128,1024] 2-bank PSUM group + 1 evac ----
        # Each emitter can be split into two half-bursts of 4 MMs; the evac
        # instruction rides with the second half.  eng: 'v' (DVE) / 's' (ACT).
        def chunk_q(b, g, eng, half=None):
            if half is None or half == 0:
                chunk_q.ps = psp.tile([128, 1024], F32, tag="ps", name=f"psq{g}_{b}")
            pq = chunk_q.ps
            cts = range(CT) if half is None else (range(2) if half == 0 else range(2, CT))
            for ct in cts:
                for mch in range(MCH):
                    nc.tensor.matmul(
                        pq[:, mch * 512 : (mch + 1) * 512],
                        wq_s[ct][:, g * 128 : (g + 1) * 128],
                        xo_s[b][ct][:, mch * 512 : (mch + 1) * 512],
                        start=(ct == 0),
                        stop=(ct == CT - 1),
                    )
            if half is None or half == 1:
                if eng == "v":
                    nc.vector.tensor_scalar_add(qT_s[b][g][:], pq[:], qb_s[g][:])
                else:
                    nc.scalar.activation(qT_s[b][g][:], pq[:], AF.Identity, bias=qb_s[g][:])

        def chunk_k(b, g, eng, half=None):
            if half is None or half == 0:
                chunk_k.ps = psp.tile([128, 1024], F32, tag="ps", name=f"psk{g}_{b}")
            pk = chunk_k.ps
            cts = range(CT) if half is None else (range(2) if half == 0 else range(2, CT))
            for ct in cts:
                for mch in range(MCH):
                    nc.tensor.matmul(
                        pk[:, mch * 512 : (mch + 1) * 512],
                        wk_s[ct][:, g * 128 : (g + 1) * 128],
                        xa_s[b][ct][:, mch * 512 : (mch + 1) * 512],
                        start=(ct == 0),
                        stop=(ct == CT - 1),
                    )
            if half is None or half == 1:
                if eng == "v":
                    nc.vector.tensor_scalar_add(kT_s[b][g][:], pk[:], kb_s[g][:])
                else:
                    nc.scalar.activation(kT_s[b][g][:], pk[:], AF.Identity, bias=kb_s[g][:])

        def chunk_v(b, jj, eng, half=None):
            if half is None or half == 0:
                chunk_v.ps = psp.tile([128, 1024], F32, tag="ps", name=f"psv{jj}_{b}")
            pv = chunk_v.ps
            cts = range(CT) if half is None else (range(2) if half == 0 else range(2, CT))
            for ct in cts:
                for jh in range(2):
                    j = jj * 2 + jh
                    nc.tensor.matmul(
                        pv[:, jh * 512 : (jh + 1) * 512],
                        xa_s[b][ct][:, j * 128 : (j + 1) * 128],
                        wv_s[ct][:],
                        start=(ct == 0),
                        stop=(ct == CT - 1),
                    )
            if half is None or half == 1:
                dst = vall[b][:, jj * 1024 : (jj + 1) * 1024]
                if eng == "v":
                    nc.vector.tensor_copy(dst, pv[:])
                else:
                    nc.scalar.copy(dst, pv[:])

        def chunk_proj(b, mt, oh, eng, half=None):
            if half is None or half == 0:
                chunk_proj.ps = psp.tile([128, 1024], F32, tag="ps", name=f"pso{mt}{oh}_{b}")
            po = chunk_proj.ps
            gs = range(G) if half is None else (range(2) if half == 0 else range(2, G))
            for g in gs:
                for oc in range(2):
                    nc.tensor.matmul(
                        po[:, oc * 512 : (oc + 1) * 512],
                        svT[b][g][:, mt * 128 : (mt + 1) * 128],
                        wp_s[g][:, oh * 1024 + oc * 512 : oh * 1024 + (oc + 1) * 512],
                        start=(g == 0),
                        stop=(g == G - 1),
                    )
            if half is None or half == 1:
                ost = ostp.tile([128, 1024], BF, tag="ost", name=f"ost{mt}{oh}_{b}")
                if has_cout:
                    nc.vector.tensor_add(ost[:], po[:], cout_s[:, oh * 1024 : (oh + 1) * 1024])
                elif eng == "v":
                    nc.vector.tensor_copy(ost[:], po[:])
                else:
                    nc.scalar.copy(ost[:], po[:])
                nc.sync.dma_start(
                    out[b, mt * 128 : (mt + 1) * 128, oh * 1024 : (oh + 1) * 1024], ost[:]
                )

        def halves(f):
            """Split one chunk emitter into two slot-sized bursts."""
            return [lambda: f(0), lambda: f(1)]

        # ---- attention unit ----
        def emit_unit(b, g, slots, carry_in):
            """slots: list of 8 filler-callable-lists, one per j-step.
            carry_in: deferred denominator work of the previous unit.
            Returns this unit's deferred work (den MMs + recip + svmul)."""
            pT = [ptp.tile([128, T], BF, tag=f"p{j}", name=f"pT{j}_{g}_{b}") for j in range(TT)]
            t1 = [trp.tile([128, T], BF, tag=f"t1{p}", name=f"t1{p}_{g}_{b}") for p in range(4)]
            t2 = [trp.tile([128, T], BF, tag=f"t2{p}", name=f"t2{p}_{g}_{b}") for p in range(2)]
            t3 = trp.tile([128, T], BF, tag="t3", name=f"t3_{g}_{b}")
            pv2 = pvp.tile([128, 1024], F32, tag="pv", name=f"pssv_{g}_{b}")
            rden = rdnp.tile([128, T], F32, tag="rd", name=f"rden{g}_{b}")

            def s_step(j):
                ps = psp.tile([128, 1024], F32, tag="ps", name=f"pss{j}_{g}_{b}")
                for mch in range(MCH):
                    nc.tensor.matmul(
                        ps[:, mch * 512 : (mch + 1) * 512],
                        kT_s[b][g][:, j * 128 : (j + 1) * 128],
                        qT_s[b][g][:, mch * 512 : (mch + 1) * 512],
                        start=True,
                        stop=True,
                    )
                nc.scalar.activation(pT[j][:], ps[:], AF.Exp)

            def sv_step(j):
                for mch in range(MCH):
                    nc.tensor.matmul(
                        pv2[:, mch * 512 : (mch + 1) * 512],
                        vall[b][:, j * 512 + g * 128 : j * 512 + (g + 1) * 128],
                        pT[j][:, mch * 512 : (mch + 1) * 512],
                        start=(j == 0),
                        stop=(j == TT - 1),
                    )

            def run(fs):
                for f in fs:
                    f()

            s_step(0)
            run(slots[0])
            s_step(1)
            run(slots[1])
            for f in carry_in[:1]:
                f()  # previous unit's SV tail (exp(6)/exp(7) latency cover)
            s_step(2)
            nc.vector.tensor_add(t1[0][:], pT[0][:], pT[1][:])
            for f in carry_in[1:]:
                f()  # previous unit's denominator + normalize
            run(slots[2])
            s_step(3)
            run(slots[3])
            s_step(4)
            nc.vector.tensor_add(t1[1][:], pT[2][:], pT[3][:])
            run(slots[4])
            s_step(5)
            run(slots[5])
            sv_step(0)
            s_step(6)
            nc.vector.tensor_add(t1[2][:], pT[4][:], pT[5][:])
            nc.vector.tensor_add(t2[0][:], t1[0][:], t1[1][:])
            run(slots[6])
            sv_step(1)
            s_step(7)
            nc.vector.tensor_add(t1[3][:], pT[6][:], pT[7][:])
            run(slots[7])
            sv_step(2)
            sv_step(3)
            sv_step(4)

            def sv_tail():
                sv_step(5)
                sv_step(6)
                sv_step(7)
                nc.vector.tensor_add(t2[1][:], t1[2][:], t1[3][:])
                nc.vector.tensor_add(t3[:], t2[0][:], t2[1][:])

            def deferred():
                pden = psp.tile([128, 1024], F32, tag="ps", name=f"psd_{g}_{b}")
                for mch in range(MCH):
                    nc.tensor.matmul(
                        pden[:, mch * 512 : (mch + 1) * 512],
                        ones_s[:],
                        t3[:, mch * 512 : (mch + 1) * 512],
                        start=True,
                        stop=True,
                    )
                nc.vector.reciprocal_approx_fast(rden[:], pden[:])
                nc.vector.tensor_mul(svT[b][g][:], pv2[:], rden[:])

            return [sv_tail, deferred]

        # ---- whole-kernel schedule ----
        def fq(b, g, e):
            return halves(lambda h: chunk_q(b, g, e, h))

        def fk(b, g, e):
            return halves(lambda h: chunk_k(b, g, e, h))

        def fv(b, jj, e):
            return halves(lambda h: chunk_v(b, jj, e, h))

        def fp(b, a, e):
            return halves(lambda h: chunk_proj(b, a // 2, a % 2, e, h))

        def whole(hs):
            h0, h1 = hs
            return [lambda: (h0(), h1())[0]]

        # prologue: Q00/Q01/Q02 interleaved at ct-granularity so each
        # arriving (xo_ct, wq_ct) DMA tile enables 3 matmul pairs instead of
        # one (the input ramp delivers a ct-pair every ~1.2us); then K00.
        pq3 = [
            psp.tile([128, 1024], F32, tag="ps", name=f"psq{g}_p0")
            for g in range(3)
        ]
        for ct in range(CT):
            for g in range(3):
                for mch in range(MCH):
                    nc.tensor.matmul(
                        pq3[g][:, mch * 512 : (mch + 1) * 512],
                        wq_s[ct][:, g * 128 : (g + 1) * 128],
                        xo_s[0][ct][:, mch * 512 : (mch + 1) * 512],
                        start=(ct == 0),
                        stop=(ct == CT - 1),
                    )
        for g, eng in ((0, "s"), (1, "v"), (2, "s")):
            if eng == "v":
                nc.vector.tensor_scalar_add(qT_s[0][g][:], pq3[g][:], qb_s[g][:])
            else:
                nc.scalar.activation(qT_s[0][g][:], pq3[g][:], AF.Identity, bias=qb_s[g][:])
        chunk_k(0, 0, "v")

        # unit (0,0) carries the rest of the batch-0 QKV (whole chunks: the
        # early ones double as DMA-ramp cover, the V chunks feed its own SV)
        carry = emit_unit(0, 0, [
            whole(fq(0, 3, "s")),
            whole(fk(0, 1, "v")),
            whole(fk(0, 2, "s")),
            whole(fk(0, 3, "v")),
            whole(fv(0, 0, "s")),
            whole(fv(0, 1, "v")),
            whole(fv(0, 2, "s")),
            whole(fv(0, 3, "v")),
        ], [])

        def half_slots(pairs):
            """4 chunk-halves pairs -> 8 slots."""
            out = []
            for h0, h1 in pairs:
                out.append([h0])
                out.append([h1])
            return out

        carry = emit_unit(0, 1, half_slots([
            fq(1, 0, "s"), fq(1, 1, "v"), fq(1, 2, "v"), fq(1, 3, "v"),
        ]), carry)
        carry = emit_unit(0, 2, half_slots([
            fk(1, 0, "s"), fk(1, 1, "v"), fk(1, 2, "v"), fk(1, 3, "v"),
        ]), carry)
        carry = emit_unit(0, 3, half_slots([
            fv(1, 0, "s"), fv(1, 1, "v"), fv(1, 2, "v"), fv(1, 3, "v"),
        ]), carry)

        def whole_late(chunks):
            """4 whole-chunk fillers at slots 3..6: late enough that the
            previous unit's deferred svT (ready ~s3) is available."""
            return [[], [], [], [chunks[0]], [chunks[1]], [chunks[2]], [chunks[3]], []]

        carry = emit_unit(1, 0, whole_late([
            whole(fp(0, 0, "s"))[0], whole(fp(0, 1, "v"))[0],
            whole(fp(0, 2, "s"))[0], whole(fp(0, 3, "v"))[0],
        ]), carry)
        carry = emit_unit(1, 1, whole_late([
            whole(fp(0, 4, "s"))[0], whole(fp(0, 5, "v"))[0],
            whole(fp(0, 6, "s"))[0], whole(fp(0, 7, "v"))[0],
        ]), carry)
        carry = emit_unit(1, 2, whole_late([
            whole(fp(0, 8, "s"))[0], whole(fp(0, 9, "v"))[0],
            whole(fp(0, 10, "s"))[0], whole(fp(0, 11, "v"))[0],
        ]), carry)
        carry = emit_unit(1, 3, whole_late([
            whole(fp(0, 12, "s"))[0], whole(fp(0, 13, "v"))[0],
            whole(fp(0, 14, "s"))[0], whole(fp(0, 15, "v"))[0],
        ]), carry)
        # epilogue: drain the last unit's deferred work, then batch-1 proj
        carry[0]()
        carry[1]()
        for a in range(15):
            chunk_proj(1, a // 2, a % 2, "vs"[a % 2])
        # last chunk: evacuate + DMA in halves so the final output transfer
        # (which nothing can hide) is half-sized
        po = psp.tile([128, 1024], F32, tag="ps", name="pso_last")
        for g in range(G):
            for oc in range(2):
                nc.tensor.matmul(
                    po[:, oc * 512 : (oc + 1) * 512],
                    svT[1][g][:, 7 * 128 : 8 * 128],
                    wp_s[g][:, 1024 + oc * 512 : 1024 + (oc + 1) * 512],
                    start=(g == 0),
                    stop=(g == G - 1),
                )
        ost_l = ostp.tile([128, 1024], BF, tag="ost", name="ost_last")
        for oc in range(2):
            osl = ost_l[:, oc * 512 : (oc + 1) * 512]
            if has_cout:
                nc.vector.tensor_add(osl, po[:, oc * 512 : (oc + 1) * 512],
                                     cout_s[:, 1024 + oc * 512 : 1024 + (oc + 1) * 512])
            elif oc == 0:
                nc.scalar.copy(osl, po[:, 0:512])
            else:
                nc.vector.tensor_copy(osl, po[:, 512:1024])
            nc.sync.dma_start(
                out[1, 7 * 128 : 8 * 128, 1024 + oc * 512 : 1024 + (oc + 1) * 512], osl
            )


def _build(has_cout):
    nc = bacc.Bacc(
        "TRN2", target_bir_lowering=False, debug=False, num_devices=N_CORES
    )
    with tile.TileContext(nc) as tc:
        _emit(tc, has_cout)
    nc.compile()
    return nc


def get_nc(has_cout=False):
    key = ("nc", has_cout)
    if key not in _CACHE:
        _CACHE[key] = _build(has_cout)
    return _CACHE[key]


def prep_inputs(attention, op_param, q_w, q_b, k_w, k_b, v_w, v_b, proj_w, proj_b):
    """Host-side layout prep: fold scales, transpose, cast to bf16, shard."""
    bf16 = ml_dtypes.bfloat16
    f32 = np.float32

    att = np.asarray(attention, f32)
    op = np.asarray(op_param, f32)

    # (n, t, c) -> (n, c, t) -> (n, 128, CT, t), bf16
    def relay_x(x):
        xt = x.transpose(0, 2, 1).reshape(x.shape[0], CT, 128, x.shape[1])
        return np.ascontiguousarray(xt.transpose(0, 2, 1, 3)).astype(bf16)

    xt_att = relay_x(att)
    xt_op = relay_x(op)

    wqT = np.ascontiguousarray(
        (np.asarray(q_w, f32) * (EQ_SCALE * ATTN_SCALE)).T
    ).astype(bf16)
    wkT = np.ascontiguousarray((np.asarray(k_w, f32) * EQ_SCALE).T).astype(bf16)

    def relay_w(wT, ncol):
        # [C, ncol] -> [128, CT*ncol] with ct-tile i at cols [i*ncol,(i+1)*ncol)
        w4 = wT.reshape(CT, 128, ncol).transpose(1, 0, 2).reshape(128, CT * ncol)
        return np.ascontiguousarray(w4).astype(bf16)

    wvT = relay_w(np.asarray(v_w, f32).T * EQ_SCALE, P)
    # proj is also an EqualLinear: weight scale 1/sqrt(PLANES) = EQ_SCALE
    wp_scaled = np.asarray(proj_w, f32) * EQ_SCALE
    wpT = relay_w(wp_scaled.T, O)

    qb2 = np.ascontiguousarray(
        (np.asarray(q_b, f32) * ATTN_SCALE).reshape(G, 128).T
    )
    kb2 = np.ascontiguousarray(np.asarray(k_b, f32).reshape(G, 128).T)
    # sum_l sim = 1, so v_b contributes proj_w @ v_b to every output row
    cout = wp_scaled @ np.asarray(v_b, f32) + np.asarray(proj_b, f32)
    has_cout = bool(np.any(cout != 0.0))
    coutb = np.ascontiguousarray(np.broadcast_to(cout[None, :], (128, O))).astype(f32)

    in_maps = []
    for core in range(N_CORES):
        lo, hi = core * B, (core + 1) * B
        m = {
            "xt_op": np.ascontiguousarray(xt_op[lo:hi]),
            "xt_att": np.ascontiguousarray(xt_att[lo:hi]),
            "wqT": wqT,
            "wkT": wkT,
            "wvT": wvT,
            "wpT": wpT,
            "qb": qb2,
            "kb": kb2,
        }
        if has_cout:
            m["coutb"] = coutb
        in_maps.append(m)
    return in_maps


def run(in_maps, trace=False, **kw):
    has_cout = "coutb" in in_maps[0]
    nc = get_nc(has_cout)
    res = run_bass_kernel_spmd(nc, in_maps, list(range(N_CORES)), trace=trace, **kw)
    return res


def kernel(**inputs) -> np.ndarray:
    in_maps = prep_inputs(**inputs)
    res = run(in_maps)
    out = np.concatenate([res.results[i]["out"] for i in range(N_CORES)], axis=0)
    return out.astype(np.float32)
